# revision 18
# baseline (speedup 1.0000x reference)
"""Trainium2 Bass kernel for: out = l2norm(relu(x@W1+b1) @ W2 + b2).

Sharding: pure data parallel over the batch dim across 8 NeuronCores.

Layout strategy:
- Host pre-transposes x so the contraction dim (64 features) sits on SBUF
  partitions. Two batch halves are packed into the 128 partitions (features
  of half A on partitions 0-63, half B on 64-127).
- mm1 uses a block-diagonal stationary [[W1,0],[0,W1]] so a single K=128
  matmul computes h.T for both halves at once (the zeros kill cross-half
  terms). ReLU+b1 is applied with a per-partition bias in transposed space.
- mm2 runs "direct": each 128-column block of h.T is the *stationary*
  operand and a block-diagonal [[W2,0],[0,W2]] (128x256) is the moving one,
  emitting e = h@W2 for an A-block and a B-block natural-layout
  [128 rows x 128 feat] pair per matmul. No transposes anywhere.
- All 8 blocks of a supertile land in one [128, 1024] PSUM region
  (A0 B0 A1 B1 ...), so the row-wise L2 norm runs as single wide ops:
  ACT Square -> DVE 3D-AP reduce -> ACT Sqrt(+eps) -> DVE reciprocal ->
  DVE broadcast multiply (stride-0 AP).

b1 is applied always (free, per-partition bias in transposed space). b2 is
per-feature in natural layout which has no cheap bias path; setup_inputs
fixes b2 = 0, so kernel() checks at call time and only compiles the b2
seeding matmuls when b2 is nonzero.

Self-contained: hardcodes all shapes; reads no sibling files.
"""

from contextlib import ExitStack

import numpy as np

import concourse.bass as bass
import concourse.tile as tile
from concourse import bacc, mybir
from concourse.bass_utils import run_bass_kernel_spmd

F32 = mybir.dt.float32

N_CORES = 8
B = 1_000_000
FEAT = 64
EMB = 128

TILE = 512            # batch rows per half-tile (mm1 moving dim)
SUPER = 2 * TILE      # rows per supertile (A half + B half)
TILES_PER_CHUNK = 8   # supertiles per DMA chunk
CHUNK = SUPER * TILES_PER_CHUNK  # 8192 batch rows per input DMA chunk
N_CHUNKS = 16
PC = CHUNK * N_CHUNKS  # 131072 rows per core (8*PC = 1048576 >= B)

EPS = 1e-20  # zero-norm guard: 1/sqrt(ssq + EPS); zero rows -> 0 output

N_BLK = TILE // 128       # 4 blocks per half
PN_W = 2 * N_BLK * EMB    # 1024: A0 B0 A1 B1 A2 B2 A3 B3 (128 feat each)


def build_bass(pc_rows: int = PC, chunk: int = CHUNK, with_b2: bool = False):
    """Build the single-core Bass program (SPMD: same program on all cores)."""
    n_chunks = pc_rows // chunk
    tiles_per_chunk = chunk // SUPER
    assert pc_rows == n_chunks * chunk and chunk == tiles_per_chunk * SUPER
    half = chunk // 2

    nc = bacc.Bacc("TRN2", target_bir_lowering=False)

    xT = nc.dram_tensor("xT", [FEAT, pc_rows], F32, kind="ExternalInput")
    w1 = nc.dram_tensor("W1", [FEAT, FEAT], F32, kind="ExternalInput")
    b1 = nc.dram_tensor("b1", [FEAT], F32, kind="ExternalInput")
    w2 = nc.dram_tensor("W2", [FEAT, EMB], F32, kind="ExternalInput")
    b2 = nc.dram_tensor("b2", [EMB], F32, kind="ExternalInput")
    out = nc.dram_tensor("out", [pc_rows, EMB], F32, kind="ExternalOutput")

    with tile.TileContext(nc) as tc, ExitStack() as ctx:
        consts = ctx.enter_context(tc.tile_pool(name="consts", bufs=1))
        xpool = ctx.enter_context(tc.tile_pool(name="xc", bufs=2))
        work = ctx.enter_context(tc.tile_pool(name="work", bufs=3))
        opool = ctx.enter_context(tc.tile_pool(name="outs", bufs=3))
        psum = ctx.enter_context(tc.tile_pool(name="psum", bufs=2, space="PSUM"))

        # Block-diagonal stationary for mm1: [[W1, 0], [0, W1]] (128x128).
        w1d = consts.tile([128, 128], F32)
        nc.vector.memset(w1d[:], 0.0)
        nc.sync.dma_start(w1d[0:FEAT, 0:FEAT], w1[:, :])
        nc.sync.dma_start(w1d[FEAT:128, FEAT:128], w1[:, :])
        # Block-diagonal moving for mm2: [[W2, 0], [0, W2]] (128x256).
        w2d = consts.tile([128, 2 * EMB], F32)
        nc.vector.memset(w2d[:], 0.0)
        nc.sync.dma_start(w2d[0:FEAT, 0:EMB], w2[:, :])
        nc.sync.dma_start(w2d[FEAT:128, EMB : 2 * EMB], w2[:, :])
        b1s = consts.tile([128, 1], F32)
        nc.sync.dma_start(b1s[0:FEAT, :], b1[:].rearrange("(f o) -> f o", o=1))
        nc.sync.dma_start(b1s[FEAT:128, :], b1[:].rearrange("(f o) -> f o", o=1))
        epss = consts.tile([128, 1], F32)
        nc.vector.memset(epss[:], EPS)
        if with_b2:
            # b2 replicated as a row per PSUM bank, + a row of ones, for K=1
            # matmuls that seed the e accumulators with b2 per feature block.
            b2row = consts.tile([1, 4 * EMB], F32)
            for j in range(4):
                nc.sync.dma_start(
                    b2row[:, j * EMB : (j + 1) * EMB],
                    b2[:].rearrange("(o f) -> o f", o=1),
                )
            onesr = consts.tile([1, 128], F32)
            nc.vector.memset(onesr[:], 1.0)

        for c in range(n_chunks):
            # chunk covers rows [c*chunk, (c+1)*chunk); half A = first
            # `half` rows on partitions 0-63, half B = second on 64-127.
            xc = xpool.tile([128, half], F32, tag="xc")
            nc.sync.dma_start(
                xc[0:FEAT, :], xT[:, c * chunk : c * chunk + half]
            )
            nc.sync.dma_start(
                xc[FEAT:128, :], xT[:, c * chunk + half : (c + 1) * chunk]
            )

            for t in range(tiles_per_chunk):
                # h.T = relu(W1.T @ x.T + b1), both halves in one matmul.
                ph = psum.tile([128, TILE], F32, tag="ph")
                nc.tensor.matmul(
                    ph[:], w1d[:], xc[:, t * TILE : (t + 1) * TILE],
                    start=True, stop=True,
                )
                hts = work.tile([128, TILE], F32, tag="hts")
                nc.scalar.activation(
                    hts[:], ph[:], mybir.ActivationFunctionType.Relu, bias=b1s[:]
                )

                # e = h @ W2 (+ b2) in natural layout. Block j produces the
                # A-block and B-block pair side by side: pn free layout is
                # [A_j (128 feat) | B_j (128 feat)] for j = 0..3.
                pn = psum.tile([128, PN_W], F32, tag="pn")
                if with_b2:
                    for bank in range(2):
                        nc.tensor.matmul(
                            pn[:, bank * 512 : (bank + 1) * 512],
                            onesr[:], b2row[:],
                            start=True, stop=False,
                        )
                for j in range(N_BLK):
                    nc.tensor.matmul(
                        pn[:, j * 2 * EMB : (j + 1) * 2 * EMB],
                        hts[:, j * 128 : (j + 1) * 128],
                        w2d[:],
                        start=not with_b2, stop=True,
                    )

                # row L2 norm; rows on partitions, features on free dim.
                esq = work.tile([128, PN_W], F32, tag="esq")
                nc.scalar.activation(
                    esq[:], pn[:], mybir.ActivationFunctionType.Square
                )
                ssq = work.tile([128, 2 * N_BLK], F32, tag="ssq")
                nc.vector.reduce_sum(
                    ssq[:],
                    esq[:].rearrange("p (j f) -> p j f", f=EMB),
                    axis=mybir.AxisListType.X,
                )
                nrm = work.tile([128, 2 * N_BLK], F32, tag="nrm")
                nc.scalar.activation(
                    nrm[:], ssq[:], mybir.ActivationFunctionType.Sqrt, bias=epss[:]
                )
                rinv = work.tile([128, 2 * N_BLK], F32, tag="rinv")
                nc.vector.reciprocal(rinv[:], nrm[:])

                outs = opool.tile([128, PN_W], F32, tag="outs")
                nc.vector.tensor_tensor(
                    outs[:].rearrange("p (j f) -> p j f", f=EMB),
                    pn[:].rearrange("p (j f) -> p j f", f=EMB),
                    rinv[:]
                    .rearrange("p (j o) -> p j o", o=1)
                    .to_broadcast((128, 2 * N_BLK, EMB)),
                    op=mybir.AluOpType.mult,
                )

                # store. outs blocks alternate A_j | B_j; A rows at
                # rA + j*128 + p, B rows at rB + j*128 + p.
                rA = c * chunk + t * TILE
                rB = c * chunk + half + t * TILE
                outs3 = outs[:].rearrange("p (j hf) -> p j hf", hf=2 * EMB)
                for hh, r0 in ((0, rA), (1, rB)):
                    nc.sync.dma_start(
                        out[r0 : r0 + TILE, :].rearrange(
                            "(j p) f -> p j f", p=128
                        ),
                        outs3[:, :, hh * EMB : (hh + 1) * EMB],
                    )

    nc.compile()
    return nc


def _prep_inputs(x, W1, b1, W2, b2, pc_rows=PC):
    total = N_CORES * pc_rows
    xp = np.zeros((total, FEAT), dtype=np.float32)
    xp[:B] = np.asarray(x, dtype=np.float32)
    w1 = np.ascontiguousarray(np.asarray(W1, dtype=np.float32))
    b1 = np.ascontiguousarray(np.asarray(b1, dtype=np.float32))
    w2 = np.ascontiguousarray(np.asarray(W2, dtype=np.float32))
    b2 = np.ascontiguousarray(np.asarray(b2, dtype=np.float32))
    in_maps = []
    for c in range(N_CORES):
        shard = xp[c * pc_rows : (c + 1) * pc_rows]
        in_maps.append(
            {
                "xT": np.ascontiguousarray(shard.T),
                "W1": w1,
                "b1": b1,
                "W2": w2,
                "b2": b2,
            }
        )
    return in_maps


_CACHED = {}


def _get_bass(with_b2: bool = False):
    key = ("nc", with_b2)
    if key not in _CACHED:
        _CACHED[key] = build_bass(with_b2=with_b2)
    return _CACHED[key]


def kernel(x, W1, b1, W2, b2, trace=False):
    with_b2 = bool(np.any(np.asarray(b2)))
    nc = _get_bass(with_b2=with_b2)
    in_maps = _prep_inputs(x, W1, b1, W2, b2)
    res = run_bass_kernel_spmd(
        nc, in_maps, core_ids=list(range(N_CORES)), trace=trace
    )
    out = np.concatenate([r["out"] for r in res.results], axis=0)[:B]
    if trace:
        kernel.last_exec_time_ns = res.exec_time_ns
        kernel.last_results = res
    return out


# revision 28
# speedup vs baseline: 1.0273x; 1.0273x over previous
"""Trainium2 Bass kernel for: out = l2norm(relu(x@W1+b1) @ W2 + b2).

Sharding: pure data parallel over the batch dim across 8 NeuronCores.

Layout strategy:
- Host pre-transposes x so the contraction dim (64 features) sits on SBUF
  partitions. Two batch halves are packed into the 128 partitions (features
  of half A on partitions 0-63, half B on 64-127).
- mm1 uses a block-diagonal stationary [[W1,0],[0,W1]] so a single K=128
  matmul computes h.T for both halves at once (the zeros kill cross-half
  terms). ReLU+b1 is applied with a per-partition bias in transposed space.
- mm2 runs "direct": each 128-column block of h.T is the *stationary*
  operand and a block-diagonal [[W2,0],[0,W2]] (128x256) is the moving one,
  emitting e = h@W2 for an A-block and a B-block natural-layout
  [128 rows x 128 feat] pair per matmul. No transposes anywhere.
- All 8 blocks of a supertile land in one [128, 1024] PSUM region
  (A0 B0 A1 B1 ...), so the row-wise L2 norm runs as single wide ops:
  ACT Square -> DVE 3D-AP reduce -> ACT Sqrt(+eps) -> DVE reciprocal ->
  DVE broadcast multiply (stride-0 AP).

b1 is applied always (free, per-partition bias in transposed space). b2 is
per-feature in natural layout which has no cheap bias path; setup_inputs
fixes b2 = 0, so kernel() checks at call time and only compiles the b2
seeding matmuls when b2 is nonzero.

Self-contained: hardcodes all shapes; reads no sibling files.
"""

from contextlib import ExitStack

import numpy as np

import concourse.bass as bass
import concourse.tile as tile
from concourse import bacc, mybir
from concourse.bass_utils import run_bass_kernel_spmd

F32 = mybir.dt.float32

N_CORES = 8
B = 1_000_000
FEAT = 64
EMB = 128

TILE = 512            # batch rows per half-tile (mm1 moving dim)
SUPER = 2 * TILE      # rows per supertile (A half + B half)
TILES_PER_CHUNK = 16  # supertiles per DMA chunk
CHUNK = SUPER * TILES_PER_CHUNK  # 16384 batch rows per input DMA chunk
N_CHUNKS = 8
PC = CHUNK * N_CHUNKS  # 131072 rows per core (8*PC = 1048576 >= B)

EPS = 1e-20  # zero-norm guard: 1/sqrt(ssq + EPS); zero rows -> 0 output

N_BLK = TILE // 128       # 4 blocks per half
PN_W = 2 * N_BLK * EMB    # 1024: A0 B0 A1 B1 A2 B2 A3 B3 (128 feat each)


def build_bass(
    pc_rows: int = PC,
    chunk: int = CHUNK,
    with_b2: bool = False,
    mm_f32r: bool = False,
    pn_bufs: int = 3,
    ph_bufs: int = 2,
    work_bufs: int = 3,
    out_bufs: int = 3,
    x_bufs: int = 2,
    out_batch: int = 1,
    norm_path: str = "wide",
):
    """Build the single-core Bass program (SPMD: same program on all cores).

    mm_f32r: run both matmuls in float32r (TF32-like, ~3x faster on the PE,
    ~1e-4 relative error) instead of full fp32.
    """
    n_chunks = pc_rows // chunk
    tiles_per_chunk = chunk // SUPER
    assert pc_rows == n_chunks * chunk and chunk == tiles_per_chunk * SUPER
    half = chunk // 2
    MMDT = mybir.dt.float32r if mm_f32r else F32

    nc = bacc.Bacc("TRN2", target_bir_lowering=False)

    xT = nc.dram_tensor("xT", [FEAT, pc_rows], F32, kind="ExternalInput")
    w1 = nc.dram_tensor("W1", [FEAT, FEAT], F32, kind="ExternalInput")
    b1 = nc.dram_tensor("b1", [FEAT], F32, kind="ExternalInput")
    w2 = nc.dram_tensor("W2", [FEAT, EMB], F32, kind="ExternalInput")
    b2 = nc.dram_tensor("b2", [EMB], F32, kind="ExternalInput")
    out = nc.dram_tensor("out", [pc_rows, EMB], F32, kind="ExternalOutput")

    with tile.TileContext(nc) as tc, ExitStack() as ctx:
        consts = ctx.enter_context(tc.tile_pool(name="consts", bufs=1))
        xpool = ctx.enter_context(tc.tile_pool(name="xc", bufs=x_bufs))
        work = ctx.enter_context(tc.tile_pool(name="work", bufs=work_bufs))
        opool = ctx.enter_context(tc.tile_pool(name="outs", bufs=out_bufs))
        psum = ctx.enter_context(tc.tile_pool(name="psum", bufs=2, space="PSUM"))
        psum2 = ctx.enter_context(tc.tile_pool(name="psum2", bufs=pn_bufs, space="PSUM"))

        # Block-diagonal stationary for mm1: [[W1, 0], [0, W1]] (128x128).
        wdma = nc.gpsimd if mm_f32r else nc.sync  # SWDGE casts f32 -> f32r
        w1d = consts.tile([128, 128], MMDT)
        nc.vector.memset(w1d[:].bitcast(F32), 0.0)
        wdma.dma_start(w1d[0:FEAT, 0:FEAT], w1[:, :])
        wdma.dma_start(w1d[FEAT:128, FEAT:128], w1[:, :])
        # Block-diagonal moving for mm2: [[W2, 0], [0, W2]] (128x256).
        w2d = consts.tile([128, 2 * EMB], MMDT)
        nc.vector.memset(w2d[:].bitcast(F32), 0.0)
        wdma.dma_start(w2d[0:FEAT, 0:EMB], w2[:, :])
        wdma.dma_start(w2d[FEAT:128, EMB : 2 * EMB], w2[:, :])
        b1s = consts.tile([128, 1], F32)
        nc.sync.dma_start(b1s[0:FEAT, :], b1[:].rearrange("(f o) -> f o", o=1))
        nc.sync.dma_start(b1s[FEAT:128, :], b1[:].rearrange("(f o) -> f o", o=1))
        epss = consts.tile([128, 1], F32)
        nc.vector.memset(epss[:], EPS)
        if with_b2:
            # b2 replicated as a row per PSUM bank, + a row of ones, for K=1
            # matmuls that seed the e accumulators with b2 per feature block.
            b2row = consts.tile([1, 4 * EMB], F32)
            for j in range(4):
                nc.sync.dma_start(
                    b2row[:, j * EMB : (j + 1) * EMB],
                    b2[:].rearrange("(o f) -> o f", o=1),
                )
            onesr = consts.tile([1, 128], F32)
            nc.vector.memset(onesr[:], 1.0)

        for c in range(n_chunks):
            # chunk covers rows [c*chunk, (c+1)*chunk); half A = first
            # `half` rows on partitions 0-63, half B = second on 64-127.
            xc = xpool.tile([128, half], MMDT, tag="xc")
            wdma.dma_start(
                xc[0:FEAT, :], xT[:, c * chunk : c * chunk + half]
            )
            wdma.dma_start(
                xc[FEAT:128, :], xT[:, c * chunk + half : (c + 1) * chunk]
            )

            for t in range(tiles_per_chunk):
                # h.T = relu(W1.T @ x.T + b1), both halves in one matmul.
                ph = psum.tile([128, TILE], F32, tag="ph", bufs=ph_bufs)
                nc.tensor.matmul(
                    ph[:], w1d[:], xc[:, t * TILE : (t + 1) * TILE],
                    start=True, stop=True,
                )
                hts = work.tile([128, TILE], MMDT, tag="hts")
                nc.scalar.activation(
                    hts[:], ph[:], mybir.ActivationFunctionType.Relu, bias=b1s[:]
                )

                # e = h @ W2 (+ b2) in natural layout. Block j produces the
                # A-block and B-block pair side by side: pn free layout is
                # [A_j (128 feat) | B_j (128 feat)] for j = 0..3.
                pn = psum2.tile([128, PN_W], F32, tag="pn")
                if with_b2:
                    for bank in range(2):
                        nc.tensor.matmul(
                            pn[:, bank * 512 : (bank + 1) * 512],
                            onesr[:], b2row[:],
                            start=True, stop=False,
                        )
                for j in range(N_BLK):
                    nc.tensor.matmul(
                        pn[:, j * 2 * EMB : (j + 1) * 2 * EMB],
                        hts[:, j * 128 : (j + 1) * 128],
                        w2d[:],
                        start=not with_b2, stop=True,
                    )

                # row L2 norm; rows on partitions, features on free dim.
                esq = work.tile([128, PN_W], F32, tag="esq")
                ssq = work.tile([128, 2 * N_BLK], F32, tag="ssq")
                if norm_path == "wide":
                    nc.scalar.activation(
                        esq[:], pn[:], mybir.ActivationFunctionType.Square
                    )
                    nc.vector.reduce_sum(
                        ssq[:],
                        esq[:].rearrange("p (j f) -> p j f", f=EMB),
                        axis=mybir.AxisListType.X,
                    )
                elif norm_path == "sqacc":
                    for jj in range(2 * N_BLK):
                        nc.scalar.activation(
                            esq[:, jj * EMB : (jj + 1) * EMB],
                            pn[:, jj * EMB : (jj + 1) * EMB],
                            mybir.ActivationFunctionType.Square,
                            accum_out=ssq[:, jj : jj + 1],
                        )
                else:
                    raise ValueError(norm_path)
                nrm = work.tile([128, 2 * N_BLK], F32, tag="nrm")
                nc.scalar.activation(
                    nrm[:], ssq[:], mybir.ActivationFunctionType.Sqrt, bias=epss[:]
                )
                rinv = work.tile([128, 2 * N_BLK], F32, tag="rinv")
                nc.vector.reciprocal(rinv[:], nrm[:])

                ob = t % out_batch
                if ob == 0:
                    outs_big = opool.tile(
                        [128, out_batch * PN_W], F32, tag="outs",
                        name=f"outs_c{c}_t{t}",
                    )
                    build_bass._outs_big = outs_big
                outs_big = build_bass._outs_big
                outs = outs_big[:, ob * PN_W : (ob + 1) * PN_W]
                nc.vector.tensor_tensor(
                    outs[:].rearrange("p (j f) -> p j f", f=EMB),
                    pn[:].rearrange("p (j f) -> p j f", f=EMB),
                    rinv[:]
                    .rearrange("p (j o) -> p j o", o=1)
                    .to_broadcast((128, 2 * N_BLK, EMB)),
                    op=mybir.AluOpType.mult,
                )

                # store. outs blocks alternate A_j | B_j; A rows at
                # rA + j*128 + p, B rows at rB + j*128 + p.
                rA = c * chunk + t * TILE
                rB = c * chunk + half + t * TILE
                if ob == out_batch - 1:
                    nrows = out_batch * TILE
                    rA0 = rA - ob * TILE
                    rB0 = rB - ob * TILE
                    outs3 = outs_big[:].rearrange(
                        "p (j hf) -> p j hf", hf=2 * EMB
                    )
                    for hh, r0 in ((0, rA0), (1, rB0)):
                        nc.sync.dma_start(
                            out[r0 : r0 + nrows, :].rearrange(
                                "(j p) f -> p j f", p=128
                            ),
                            outs3[:, :, hh * EMB : (hh + 1) * EMB],
                        )

    nc.compile()
    return nc


def _prep_inputs(x, W1, b1, W2, b2, pc_rows=PC):
    total = N_CORES * pc_rows
    xp = np.zeros((total, FEAT), dtype=np.float32)
    xp[:B] = np.asarray(x, dtype=np.float32)
    w1 = np.ascontiguousarray(np.asarray(W1, dtype=np.float32))
    b1 = np.ascontiguousarray(np.asarray(b1, dtype=np.float32))
    w2 = np.ascontiguousarray(np.asarray(W2, dtype=np.float32))
    b2 = np.ascontiguousarray(np.asarray(b2, dtype=np.float32))
    in_maps = []
    for c in range(N_CORES):
        shard = xp[c * pc_rows : (c + 1) * pc_rows]
        in_maps.append(
            {
                "xT": np.ascontiguousarray(shard.T),
                "W1": w1,
                "b1": b1,
                "W2": w2,
                "b2": b2,
            }
        )
    return in_maps


_CACHED = {}


def _get_bass(with_b2: bool = False):
    key = ("nc", with_b2)
    if key not in _CACHED:
        _CACHED[key] = build_bass(with_b2=with_b2)
    return _CACHED[key]


def kernel(x, W1, b1, W2, b2, trace=False):
    with_b2 = bool(np.any(np.asarray(b2)))
    nc = _get_bass(with_b2=with_b2)
    in_maps = _prep_inputs(x, W1, b1, W2, b2)
    res = run_bass_kernel_spmd(
        nc, in_maps, core_ids=list(range(N_CORES)), trace=trace
    )
    out = np.concatenate([r["out"] for r in res.results], axis=0)[:B]
    if trace:
        kernel.last_exec_time_ns = res.exec_time_ns
        kernel.last_results = res
    return out
